# revision 84
# baseline (speedup 1.0000x reference)
"""BigBird encoder block on 8 Trainium2 NeuronCores.

Sharding: pure data-parallel over batch (B=8 -> 1 batch element per core).
All weights replicated; each core runs the full encoder block for one batch
element. No collectives.

Per-core dataflow (activations kept "transposed" with D on partitions):
  LN1 token-major -> PE-transpose -> xT [D, L] bf16 (+ raw input transpose
  for the residual). QKV as [128 = 2 heads x 64 hd, L] bf16 per head pair.
  Attention is block-sparse with host-computed attended sets S_i and inverse
  incidence Q_j: scores computed keys-on-partitions, one stationary load per
  (head, key block), N batched over runs of consecutive query blocks; exp on
  the scalar engine (no masks needed: S_i are dedup'd sets, padding is folded
  into V and the softmax-denominator ones column); AV accumulates in PSUM
  across key blocks — legal because global blocks 0 and 15 belong to every
  S_i, so every accumulation group starts at j=0 and stops at j=15.
  Then Wo + residual, LN2 (stats via ones-matmul in transposed space), MLP
  with fused bias+gelu, final PE-transpose back to token-major.
"""

import numpy as np
from contextlib import ExitStack

B, L, D = 8, 1024, 512
H, HD = 8, 64
BLK = 64
M = L // BLK   # 16
MLP = 1024
NCORES = 8
P = 128

_kernel_cache = {}


def _attended_sets(rand_attn):
    S = []
    for i in range(M):
        if i == 0 or i == M - 1:
            S.append(list(range(M)))
            continue
        s = {0, M - 1, (i - 1) % M, i, (i + 1) % M}
        for r in rand_attn[i]:
            s.add(int(r))
        S.append(sorted(s))
    return S


def _runs(ints):
    runs = []
    for x in ints:
        if runs and x == runs[-1][0] + runs[-1][1]:
            runs[-1][1] += 1
        else:
            runs.append([x, 1])
    return [(a, n) for a, n in runs]


def _runs_b(ints):
    """Runs of consecutive ints, split at 8 (PSUM bank boundary at col 512)."""
    out = []
    for (a, n) in _runs(ints):
        if a < 8 < a + n:
            out.append((a, 8 - a))
            out.append((8, a + n - 8))
        else:
            out.append((a, n))
    return out


def _chunk_sizes(n):
    out = []
    while n > 0:
        out.append(min(8, n))
        n -= 8
    return out


def _np_bf16(x):
    import ml_dtypes
    return np.asarray(x, np.float32).astype(ml_dtypes.bfloat16)


def _np_fp8(x):
    import ml_dtypes
    return np.asarray(x, np.float32).astype(ml_dtypes.float8_e4m3)


def _build_bass(S, Q, bias1, bias2):
    import concourse.bacc as bacc
    import concourse.tile as tile
    from concourse import mybir
    from concourse.masks import make_identity

    f32 = mybir.dt.float32
    bf16 = mybir.dt.bfloat16
    AF = mybir.ActivationFunctionType

    nc = bacc.Bacc("TRN2", target_bir_lowering=False)

    fp8 = mybir.dt.float8e4
    x_in = nc.dram_tensor("x_in", [L, D], bf16, kind="ExternalInput")
    wq_d = nc.dram_tensor("wq", [D, H * HD], fp8, kind="ExternalInput")
    wk_d = nc.dram_tensor("wk", [D, H * HD], fp8, kind="ExternalInput")
    wv_d = nc.dram_tensor("wv", [D, H * HD], fp8, kind="ExternalInput")
    wo_d = nc.dram_tensor("wo", [H * HD, D], fp8, kind="ExternalInput")
    w1_d = nc.dram_tensor("w1", [D, MLP], fp8, kind="ExternalInput")
    w2_d = nc.dram_tensor("w2", [MLP, D], fp8, kind="ExternalInput")
    b1_d = nc.dram_tensor("b1", [MLP], f32, kind="ExternalInput")
    b2_d = nc.dram_tensor("b2", [D], f32, kind="ExternalInput")
    if bias1:
        ln1b_d = nc.dram_tensor("ln1b", [D], f32, kind="ExternalInput")
    pad_d = nc.dram_tensor("padm", [P, M], f32, kind="ExternalInput")
    out_d = nc.dram_tensor("out", [L, D], bf16, kind="ExternalOutput")

    NT = L // P
    DC = D // P
    HP = H // 2
    MC = MLP // P
    EPS = 1e-6

    with tile.TileContext(nc) as tc, ExitStack() as ctx:
        const = ctx.enter_context(tc.tile_pool(name="const", bufs=1))
        big = ctx.enter_context(tc.tile_pool(name="big", bufs=1))
        resid = ctx.enter_context(tc.tile_pool(name="resid", bufs=2))
        tok = ctx.enter_context(tc.tile_pool(name="tok", bufs=2))
        small = ctx.enter_context(tc.tile_pool(name="small", bufs=4))
        rows = ctx.enter_context(tc.tile_pool(name="rows", bufs=1))
        attnp = ctx.enter_context(tc.tile_pool(name="attnp", bufs=10))
        vkmp = ctx.enter_context(tc.tile_pool(name="vkmp", bufs=2))
        bcast = ctx.enter_context(tc.tile_pool(name="bcast", bufs=1))
        normt = ctx.enter_context(tc.tile_pool(name="normt", bufs=2))
        psU = ctx.enter_context(tc.tile_pool(name="psU", bufs=4,
                                              space="PSUM"))
        psAV = ctx.enter_context(tc.tile_pool(name="psAV", bufs=2, space="PSUM"))

        xts = []
        for t in range(NT):
            xt = tok.tile([P, D], bf16, tag="xt", bufs=NT,
                          name=f"xt{t}")
            nc.sync.dma_start(xt[:], x_in[t * P:(t + 1) * P, :])
            xts.append(xt)

        id_bf = const.tile([P, P], bf16)
        make_identity(nc, id_bf[:])

        def load_w(dram, kdim, ndim):
            t = const.tile([P, kdim // P, ndim], fp8, tag=dram.name)
            nc.sync.dma_start(t[:], dram.rearrange("(c p) n -> p c n", p=P))
            return t

        wq = load_w(wq_d, D, H * HD)
        wk = load_w(wk_d, D, H * HD)
        wv = load_w(wv_d, D, H * HD)
        wo = load_w(wo_d, H * HD, D)
        w1 = load_w(w1_d, D, MLP)
        w2 = load_w(w2_d, MLP, D)

        b1c = const.tile([P, MC], f32)
        nc.sync.dma_start(b1c[:], b1_d.rearrange("(c p) -> p c", p=P))
        b2c = const.tile([P, DC], f32)
        nc.sync.dma_start(b2c[:], b2_d.rearrange("(c p) -> p c", p=P))
        padm = const.tile([P, M], f32)
        nc.sync.dma_start(padm[:], pad_d[:])
        if bias1:
            ln1bB = const.tile([P, D], f32)
            nc.sync.dma_start(ln1bB[:],
                              ln1b_d[None, :].to_broadcast((P, D)))
        eps_col = const.tile([P, 1], f32)
        nc.vector.memset(eps_col[:], EPS)
        # ones row at partition 64: K=1 stationary for broadcasting the
        # 1/denom rows (which live at partition 64 of each acc tile)
        selb = const.tile([65, P], bf16)
        nc.vector.memset(selb[64:65, :], 1.0)
        onesb = const.tile([1, P], bf16)
        nc.vector.memset(onesb[:], 1.0)
        ones_colb = const.tile([P, 1], bf16)
        nc.vector.memset(ones_colb[:], 1.0)

        xT = big.tile([P, DC, L], fp8, tag="xT")
        inT = resid.tile([P, DC, L], bf16, tag="res")
        qT = big.tile([P, HP, L], bf16, tag="qT")
        kT = big.tile([P, HP, L], bf16, tag="kT")
        vT = big.tile([P, HP, L], bf16, tag="vT")
        y1T = big.tile([P, MC, L], fp8, tag="y1T")
        # outT / ln2T are allocated later with tags "xT" / "qT", reusing the
        # slots of xT (dead after QKV) and qT (dead after attention).

        # ---- LN1 (token-major) + transposes ----
        # ln1_scale is folded into Wq/Wk/Wv host-side; when fold_b1 the bias
        # is b/s added pre-projection (exact), and when the bias is zero it
        # vanishes. Stats: sum(x) and sum(x^2) via Act accumulators (keeps
        # the DVE off the critical path); var = E[x^2] - mu^2.
        f32r = mybir.dt.float32r

        def ln1_tile(t):
            xt = xts[t]
            for c in range(DC):
                pf = psU.tile([P, P], bf16, tag="u", name=f"tpb{t}_{c}")
                nc.tensor.transpose(pf[:], xt[:, c * P:(c + 1) * P],
                                    id_bf[:])
                nc.vector.tensor_copy(inT[:, c, t * P:(t + 1) * P], pf[:])
            sumx = small.tile([P, 1], f32, tag="sumx")
            cpy = tok.tile([P, D], bf16, tag="cent")
            nc.vector.scalar_tensor_tensor(
                out=cpy[:], in0=xt[:], scalar=1.0, in1=xt[:],
                op0=mybir.AluOpType.mult, op1=mybir.AluOpType.max,
                accum_out=sumx[:])
            sq = tok.tile([P, D], bf16, tag="sq")
            sumsq = small.tile([P, 1], f32, tag="sumsq")
            nc.scalar.activation(sq[:], xt[:], AF.Square, accum_out=sumsq[:])
            # stats smalls kept on DVE with ONE Act hop (Sqrt) to avoid
            # per-tile cross-engine ping-pong on the in-order queues
            nmean = small.tile([P, 1], f32, tag="nmean")
            nc.vector.tensor_scalar(
                nmean[:], sumx[:], -1.0 / D, 0.0,
                op0=mybir.AluOpType.mult, op1=mybir.AluOpType.add)
            mu2 = small.tile([P, 1], f32, tag="mu2s")
            nc.vector.tensor_mul(mu2[:], nmean[:], nmean[:])
            varb = small.tile([P, 1], f32, tag="varb")
            nc.vector.scalar_tensor_tensor(
                out=varb[:], in0=sumsq[:], scalar=1.0 / D, in1=mu2[:],
                op0=mybir.AluOpType.mult, op1=mybir.AluOpType.subtract)
            rstd = small.tile([P, 1], f32, tag="rstd")
            nc.scalar.activation(rstd[:], varb[:], AF.Sqrt, bias=eps_col[:])
            nc.vector.reciprocal(rstd[:], rstd[:])
            nmr = small.tile([P, 1], f32, tag="nmr")
            nc.vector.tensor_mul(nmr[:], nmean[:], rstd[:])
            xnb = tok.tile([P, D], bf16, tag="xnb")
            with nc.allow_low_precision(reason="qkv in fp8: rel tol 2e-2"):
                if bias1:
                    xn = tok.tile([P, D], f32, tag="xn")
                    nc.vector.tensor_scalar(
                        xn[:], xt[:], rstd[:], nmr[:],
                        op0=mybir.AluOpType.mult, op1=mybir.AluOpType.add)
                    nc.vector.tensor_add(xnb[:], xn[:], ln1bB[:])
                else:
                    nc.vector.tensor_scalar(
                        xnb[:], xt[:], rstd[:], nmr[:],
                        op0=mybir.AluOpType.mult, op1=mybir.AluOpType.add)
            for c in range(DC):
                pt = psU.tile([P, P], bf16, tag="u", name=f"tpa{t}_{c}")
                nc.tensor.transpose(pt[:], xnb[:, c * P:(c + 1) * P],
                                    id_bf[:])
                with nc.allow_low_precision(reason="qkv in fp8"):
                    nc.scalar.copy(xT[:, c, t * P:(t + 1) * P], pt[:])

        # ---- QKV (V first so the vkm prebuild below overlaps K/Q work) ----
        # fp8 DoubleRow: K=512 contracts as 2 groups of (128 partitions x 2).
        # Emission is split by token half and interleaved with the LN1 tiles
        # so the nh=0 projections enter the PE queue as soon as tiles 0-3 are
        # transposed, instead of sitting behind all 64 LN1 transposes.
        DR = mybir.MatmulPerfMode.DoubleRow

        for t in range(NT):
            ln1_tile(t)
        for w_sb, dstT in ((wv, vT), (wk, kT), (wq, qT)):
            for hp in range(HP):
                for nh in range(2):
                    ps = psU.tile([P, 512], f32, tag="u")
                    for c2 in range(DC // 2):
                        nc.tensor.matmul(
                            ps[:],
                            w_sb[:, 2 * c2:2 * c2 + 2, hp * P:(hp + 1) * P],
                            xT[:, 2 * c2:2 * c2 + 2,
                               nh * 512:(nh + 1) * 512],
                            start=(c2 == 0), stop=(c2 == DC // 2 - 1),
                            perf_mode=DR)
                    if nh == 0:
                        nc.vector.tensor_copy(
                            dstT[:, hp, nh * 512:(nh + 1) * 512], ps[:])
                    else:
                        nc.scalar.copy(
                            dstT[:, hp, nh * 512:(nh + 1) * 512], ps[:])

        # ---- attention ----
        # psum row halves are indexed by he (head within pair); every matmul
        # chain into an acc tile keeps ONE base partition (HW requirement:
        # accumulating from different tile_positions crashes the device).
        # vkm prebuild for ALL head pairs (overlaps the K/Q projections):
        # vkm[he*64+p, j, 0:64] = v[key j*64+p, hd] of head 2hp+he;
        # col 64 = ones (softmax denominator; zeroed for padded keys).
        # Each 64x64 transpose targets its own partition half directly, so a
        # single batched copy moves all M blocks for both halves.
        vkms = []
        for hp in range(HP):
            vkm = vkmp.tile([P, M, 65], bf16, tag="vkm", bufs=HP,
                            name=f"vkm{hp}")
            pt = psU.tile([P, M, 64], bf16, tag="u", name=f"tpv{hp}")
            for j in range(M):
                nc.tensor.transpose(pt[0:64, j, :],
                                    vT[0:64, hp, j * 64:(j + 1) * 64],
                                    id_bf[0:64, 0:64])
                nc.tensor.transpose(pt[64:128, j, :],
                                    vT[64:128, hp, j * 64:(j + 1) * 64],
                                    id_bf[64:128, 64:128])
            # ones column is 32.0: v is x32 (fp8 weight scaling), so denom
            # and av keep the same scale and the ratio is exact
            nc.vector.tensor_copy(vkm[:, :, 0:64], pt[:, :, :])
            nc.vector.memset(vkm[:, :, 64:65], 32.0)
            for j in range(M):
                nc.vector.tensor_scalar_mul(vkm[:, j, :], vkm[:, j, :],
                                            padm[:, j:j + 1])
            vkms.append(vkm)

        outT = big.tile([P, HP, L], fp8, tag="xT")

        def make_norm(hp, acc):
            # normalize: out = acc[0:64] / acc[64]. bf16 reciprocals, then a
            # K=1 bf16 ones-matmul broadcasts each 1/d row across 64
            # partitions. he1's rows move to outT's upper half by gpsimd DMA.
            # Emission is deferred into the next hp's first chunk so the PE
            # queue has independent score work while the reciprocals land.
            def norm():
                recm = [rows.tile([65, L], bf16, tag=f"rec{he}",
                                  name=f"rec{hp}_{he}") for he in range(2)]
                with nc.allow_low_precision(reason="1/denom bf16"):
                    nc.vector.reciprocal(recm[0][64:65, :], acc[0][64:65, :])
                    nc.vector.reciprocal(recm[1][64:65, :], acc[1][64:65, :])
                nt = normt.tile([64, L], fp8, tag="nt")
                for nh in range(2):
                    sl = slice(nh * 512, (nh + 1) * 512)
                    for he in range(2):
                        recB = psU.tile([64, 512], f32, tag="u",
                                        name=f"recB{hp}_{nh}_{he}")
                        nc.tensor.matmul(recB[:], selb[64:65, 0:64],
                                         recm[he][64:65, sl],
                                         start=True, stop=True)
                        recS = bcast.tile([64, 512], f32, tag="recB",
                                          bufs=2)
                        nc.scalar.copy(recS[:], recB[:])
                        with nc.allow_low_precision(reason="attn out fp8"):
                            if he == 0:
                                nc.vector.tensor_mul(outT[0:64, hp, sl],
                                                     acc[0][0:64, sl],
                                                     recS[:])
                            else:
                                nc.vector.tensor_mul(nt[:, sl],
                                                     acc[1][0:64, sl],
                                                     recS[:])
                    nc.gpsimd.dma_start(outT[64:128, hp, sl], nt[:, sl])
            return norm

        def make_av(acc, vkm, att, chunk, j):
            # AV emission is software-pipelined one chunk behind the scores
            # so the in-order PE queue never head-of-line blocks on an exp
            def av():
                for he in range(2):
                    po = he * 64
                    col = 0
                    for (i0, n) in _runs_b(chunk):
                        nc.tensor.matmul(
                            acc[he][0:65, i0 * 64:(i0 + n) * 64],
                            vkm[po:po + 64, j, :],
                            att[po:po + 64, col * 64:(col + n) * 64],
                            start=(j == 0), stop=(j == M - 1))
                        col += n
            return av

        pending_norm = None
        pending_av = None
        for hp in range(HP):
            vkm = vkms[hp]
            acc = [psAV.tile([65, L], f32, tag="avacc", name=f"acc{hp}_{he}")
                   for he in range(2)]
            for j in range(M):
                csizes = _chunk_sizes(len(Q[j]))
                for ci, csz in enumerate(csizes):
                    s0 = 8 * ci
                    chunk = Q[j][s0:s0 + csz]
                    pssc = psU.tile([P, 512], f32, tag="u",
                                     name=f"sc{hp}_{j}_{ci}")
                    for he in range(2):
                        po = he * 64
                        col = 0
                        for (i0, n) in _runs(chunk):
                            nc.tensor.matmul(
                                pssc[po:po + 64, col * 64:(col + n) * 64],
                                kT[po:po + 64, hp, j * 64:(j + 1) * 64],
                                qT[po:po + 64, hp, i0 * 64:(i0 + n) * 64],
                                start=True, stop=True)
                            col += n
                    att = attnp.tile([P, 512], bf16, tag="att",
                                     name=f"att{hp}_{j}_{ci}")
                    # scores carry the x32 q and x32 k scalings: exp
                    # rescales by 2^-10 exactly
                    nc.scalar.activation(att[:, 0:csz * 64],
                                         pssc[:, 0:csz * 64], AF.Exp,
                                         scale=2.0 ** -10)
                    # flush order matters at hp boundaries: the previous
                    # hp's last AV must hit the PE queue before its norm
                    # broadcasts (which wait on the DVE reciprocal of acc)
                    if pending_av is not None:
                        pending_av()
                        pending_av = None
                    if pending_norm is not None:
                        pending_norm()
                        pending_norm = None
                    pending_av = make_av(acc, vkm, att, chunk, j)
            pending_norm = make_norm(hp, acc)
        pending_av()
        pending_norm()

        # ---- Wo + residual ----
        xrT = resid.tile([P, DC, L], bf16, tag="res")
        for nh in range(2):
            for dc in range(DC):
                sl = slice(nh * 512, (nh + 1) * 512)
                ps = psU.tile([P, 512], f32, tag="u")
                for c2 in range(HP // 2):
                    nc.tensor.matmul(
                        ps[:],
                        wo[:, 2 * c2:2 * c2 + 2, dc * P:(dc + 1) * P],
                        outT[:, 2 * c2:2 * c2 + 2, sl],
                        start=(c2 == 0), stop=(c2 == HP // 2 - 1),
                        perf_mode=DR)
                with nc.allow_low_precision(reason="xr bf16: tol 2e-2"):
                    nc.vector.scalar_tensor_tensor(
                        out=xrT[:, dc, sl], in0=ps[:], scalar=2.0 ** -5,
                        in1=inT[:, dc, sl],
                        op0=mybir.AluOpType.mult, op1=mybir.AluOpType.add)

        # ---- LN2 (transposed space; stats via ones-matmul) ----
        mu = rows.tile([1, L], f32, tag="mu")
        msq = rows.tile([1, L], f32, tag="msq")
        mub = rows.tile([1, L], bf16, tag="mub")
        msqb = rows.tile([1, L], bf16, tag="msqb")
        for nh in range(2):
            ps_s = psU.tile([1, 512], f32, tag="u", name=f"st_s{nh}")
            ps_q = psU.tile([1, 512], f32, tag="u", name=f"st_q{nh}")
            for dc in range(DC):
                sl = slice(nh * 512, (nh + 1) * 512)
                sqc = tok.tile([P, 512], bf16, tag="sqc")
                with nc.allow_low_precision(reason="ln2 stats in bf16"):
                    nc.scalar.activation(sqc[:], xrT[:, dc, sl], AF.Square)
                nc.tensor.matmul(ps_s[:], ones_colb[:], xrT[:, dc, sl],
                                 start=(dc == 0), stop=(dc == DC - 1))
                nc.tensor.matmul(ps_q[:], ones_colb[:], sqc[:],
                                 start=(dc == 0), stop=(dc == DC - 1))
            sl = slice(nh * 512, (nh + 1) * 512)
            nc.scalar.mul(mu[0:1, sl], ps_s[:], 1.0 / D)
            nc.scalar.mul(msq[0:1, sl], ps_q[:], 1.0 / D)
            # msq <- rstd = 1/sqrt(msq - mu^2 + eps) per half, in place
            mu2 = rows.tile([1, L], f32, tag="mu2")
            nc.scalar.activation(mu2[0:1, sl], mu[0:1, sl], AF.Square)
            nc.vector.tensor_sub(msq[0:1, sl], msq[0:1, sl], mu2[0:1, sl])
            nc.scalar.activation(msq[0:1, sl], msq[0:1, sl], AF.Sqrt,
                                 bias=eps_col[0:1, :])
            with nc.allow_low_precision(reason="ln2 stat rows bf16"):
                nc.vector.reciprocal(msqb[0:1, sl], msq[0:1, sl])
                nc.vector.tensor_copy(mub[0:1, sl], mu[0:1, sl])
        ln2T = big.tile([P, DC, L], fp8, tag="qT")
        for nh in range(2):
            sl = slice(nh * 512, (nh + 1) * 512)
            muB = psU.tile([P, 512], f32, tag="u", name=f"muB{nh}")
            rstdB = psU.tile([P, 512], f32, tag="u", name=f"rstdB{nh}")
            nc.tensor.matmul(muB[:], onesb[0:1, :], mub[0:1, sl],
                             start=True, stop=True)
            nc.tensor.matmul(rstdB[:], onesb[0:1, :], msqb[0:1, sl],
                             start=True, stop=True)
            muS = bcast.tile([P, 512], bf16, tag="muS", bufs=2)
            rstdS = bcast.tile([P, 512], bf16, tag="rstdS", bufs=2)
            with nc.allow_low_precision(reason="ln2 bcast bf16"):
                nc.scalar.copy(muS[:], muB[:])
                nc.scalar.copy(rstdS[:], rstdB[:])
            for dc in range(DC):
                t1 = tok.tile([P, 512], bf16, tag="sqc")
                with nc.allow_low_precision(reason="mlp in fp8"):
                    nc.vector.tensor_sub(t1[:], xrT[:, dc, sl], muS[:])
                    nc.vector.tensor_mul(ln2T[:, dc, sl], t1[:], rstdS[:])

        # ---- MLP (nh-outer so each half reaches the store pipeline early) ----
        # foutT/ost/out are bf16: the final sum is dominated by the f32
        # residual path, so the bf16 rounding adds ~0.1% RMS (tol is 2e-2)
        foutT = resid.tile([P, DC, L], bf16, tag="res")
        dmaq = [nc.sync, nc.scalar, nc.gpsimd]
        for nh in range(2):
            for mc in range(MC):
                ps = psU.tile([P, 512], f32, tag="u")
                for c2 in range(DC // 2):
                    nc.tensor.matmul(
                        ps[:], w1[:, 2 * c2:2 * c2 + 2, mc * P:(mc + 1) * P],
                        ln2T[:, 2 * c2:2 * c2 + 2,
                             nh * 512:(nh + 1) * 512],
                        start=(c2 == 0), stop=(c2 == DC // 2 - 1),
                        perf_mode=DR)
                with nc.allow_low_precision(reason="mlp hidden fp8"):
                    nc.scalar.activation(y1T[:, mc, nh * 512:(nh + 1) * 512],
                                         ps[:], AF.Gelu_apprx_tanh,
                                         bias=b1c[:, mc:mc + 1],
                                         scale=2.0 ** -5)
            for dc in range(DC):
                ps = psU.tile([P, 512], f32, tag="u")
                for c2 in range(MC // 2):
                    nc.tensor.matmul(
                        ps[:], w2[:, 2 * c2:2 * c2 + 2, dc * P:(dc + 1) * P],
                        y1T[:, 2 * c2:2 * c2 + 2,
                            nh * 512:(nh + 1) * 512],
                        start=(c2 == 0), stop=(c2 == MC // 2 - 1),
                        perf_mode=DR)
                sl = slice(nh * 512, (nh + 1) * 512)
                with nc.allow_low_precision(reason="final out bf16"):
                    if bias2:
                        t2 = tok.tile([P, 512], f32, tag="sqc",
                                      name=f"t2_{dc}_{nh}")
                        nc.vector.tensor_scalar(
                            t2[:], ps[:], 2.0 ** -5, b2c[:, dc:dc + 1],
                            op0=mybir.AluOpType.mult,
                            op1=mybir.AluOpType.add)
                        nc.vector.tensor_add(foutT[:, dc, sl], t2[:],
                                             xrT[:, dc, sl])
                    else:
                        nc.vector.scalar_tensor_tensor(
                            out=foutT[:, dc, sl], in0=ps[:],
                            scalar=2.0 ** -5, in1=xrT[:, dc, sl],
                            op0=mybir.AluOpType.mult,
                            op1=mybir.AluOpType.add)

            # transpose this half back to token-major, store full-width
            # [128, D] slabs (one 1KB/partition DMA per tile, queues cycled)
            for t in range(nh * NT // 2, (nh + 1) * NT // 2):
                ost = normt.tile([P, D], bf16, tag="ost")
                for dc in range(DC):
                    pf = psU.tile([P, P], bf16, tag="u",
                                   name=f"tpo{dc}_{t}")
                    nc.tensor.transpose(pf[:],
                                        foutT[:, dc, t * P:(t + 1) * P],
                                        id_bf[:])
                    if dc % 2 == 0:
                        nc.vector.tensor_copy(ost[:, dc * P:(dc + 1) * P],
                                              pf[:])
                    else:
                        nc.scalar.copy(ost[:, dc * P:(dc + 1) * P], pf[:])
                dmaq[t % 3].dma_start(out_d[t * P:(t + 1) * P, :], ost[:])

    nc.compile()
    return nc


def kernel(**inputs):
    inputs = {k: np.asarray(v) for k, v in inputs.items()}
    rand_attn = inputs["rand_attn"].astype(np.int32)
    ln1s = inputs["ln1_scale"].astype(np.float32)
    ln1b = inputs["ln1_bias"].astype(np.float32)
    ln2s = inputs["ln2_scale"].astype(np.float32)
    ln2b = inputs["ln2_bias"].astype(np.float32)
    bias1 = bool(np.any(ln1b != 0.0))
    bias2 = bool(np.any(np.asarray(inputs["b2"]) != 0.0))
    key = (rand_attn.tobytes(), bias1, bias2)
    if key not in _kernel_cache:
        S = _attended_sets(rand_attn)
        Q = [[i for i in range(M) if j in S[i]] for j in range(M)]
        _kernel_cache[key] = _build_bass(S, Q, bias1, bias2)
    nc = _kernel_cache[key]

    x = inputs["inputs"].astype(np.float32)
    pm = np.asarray(inputs["padding_mask"]).astype(np.float32)
    # ln1_scale folds into the qkv projections (rows scaled); a nonzero
    # ln1_bias becomes a pre-projection add of b/s (exact). ln2 scale AND
    # bias fold into W1/b1 exactly: gelu(ln*s2+b2c)@W1+b1 =
    # gelu(ln@(diag(s2)W1) + (b2c@W1+b1)).
    # weights are scaled x32 before fp8 quantization (their ~0.02 magnitudes
    # sit in e4m3's denormal range otherwise); every x32 is compensated
    # downstream by an exact power-of-2 rescale inside the kernel.
    s1 = np.where(ln1s == 0.0, np.float32(1e-30), ln1s)[:, None]
    wq = _np_fp8(32.0 * s1 * inputs["Wq"].reshape(D, H * HD) / np.sqrt(HD))
    wk = _np_fp8(32.0 * s1 * inputs["Wk"].reshape(D, H * HD))
    wv = _np_fp8(32.0 * s1 * inputs["Wv"].reshape(D, H * HD))
    wo = _np_fp8(32.0 * inputs["Wo"].reshape(H * HD, D))
    w1f = inputs["W1"].astype(np.float32)
    w1 = _np_fp8(32.0 * ln2s[:, None] * w1f)
    w2 = _np_fp8(32.0 * inputs["W2"])
    b1f = inputs["b1"].astype(np.float32) + ln2b @ w1f
    common = dict(
        wq=wq, wk=wk, wv=wv, wo=wo, w1=w1, w2=w2,
        b1=b1f.astype(np.float32),
        b2=inputs["b2"].astype(np.float32),
    )
    if bias1:
        common["ln1b"] = (ln1b / s1[:, 0]).astype(np.float32)
    in_maps = []
    for c in range(NCORES):
        pj = pm[c, :, 0].reshape(M, BLK).T          # [64, M]
        padm = np.concatenate([pj, pj], axis=0)      # both he row halves
        in_maps.append(dict(common, x_in=_np_bf16(x[c]),
                            padm=padm.astype(np.float32)))

    from concourse.bass_utils import run_bass_kernel_spmd
    res = run_bass_kernel_spmd(nc, in_maps, core_ids=list(range(NCORES)))
    return np.stack([np.asarray(res.results[c]["out"], np.float32)
                     for c in range(NCORES)], axis=0)



# revision 91
# speedup vs baseline: 1.0102x; 1.0102x over previous
"""BigBird encoder block on 8 Trainium2 NeuronCores.

Sharding: pure data-parallel over batch (B=8 -> 1 batch element per core).
All weights replicated; each core runs the full encoder block for one batch
element. No collectives.

Per-core dataflow (activations kept "transposed" with D on partitions):
  LN1 token-major -> PE-transpose -> xT [D, L] bf16 (+ raw input transpose
  for the residual). QKV as [128 = 2 heads x 64 hd, L] bf16 per head pair.
  Attention is block-sparse with host-computed attended sets S_i and inverse
  incidence Q_j: scores computed keys-on-partitions, one stationary load per
  (head, key block), N batched over runs of consecutive query blocks; exp on
  the scalar engine (no masks needed: S_i are dedup'd sets, padding is folded
  into V and the softmax-denominator ones column); AV accumulates in PSUM
  across key blocks — legal because global blocks 0 and 15 belong to every
  S_i, so every accumulation group starts at j=0 and stops at j=15.
  Then Wo + residual, LN2 (stats via ones-matmul in transposed space), MLP
  with fused bias+gelu, final PE-transpose back to token-major.
"""

import numpy as np
from contextlib import ExitStack

B, L, D = 8, 1024, 512
H, HD = 8, 64
BLK = 64
M = L // BLK   # 16
MLP = 1024
NCORES = 8
P = 128

_kernel_cache = {}


def _attended_sets(rand_attn):
    S = []
    for i in range(M):
        if i == 0 or i == M - 1:
            S.append(list(range(M)))
            continue
        s = {0, M - 1, (i - 1) % M, i, (i + 1) % M}
        for r in rand_attn[i]:
            s.add(int(r))
        S.append(sorted(s))
    return S


def _runs(ints):
    runs = []
    for x in ints:
        if runs and x == runs[-1][0] + runs[-1][1]:
            runs[-1][1] += 1
        else:
            runs.append([x, 1])
    return [(a, n) for a, n in runs]


def _runs_b(ints):
    """Runs of consecutive ints, split at 8 (PSUM bank boundary at col 512)."""
    out = []
    for (a, n) in _runs(ints):
        if a < 8 < a + n:
            out.append((a, 8 - a))
            out.append((8, a + n - 8))
        else:
            out.append((a, n))
    return out


def _chunk_sizes(n):
    out = []
    while n > 0:
        out.append(min(8, n))
        n -= 8
    return out


def _np_bf16(x):
    import ml_dtypes
    return np.asarray(x, np.float32).astype(ml_dtypes.bfloat16)


def _np_fp8(x):
    import ml_dtypes
    return np.asarray(x, np.float32).astype(ml_dtypes.float8_e4m3)


def _build_bass(S, Q, bias1, bias2):
    import concourse.bacc as bacc
    import concourse.tile as tile
    from concourse import mybir
    from concourse.masks import make_identity

    f32 = mybir.dt.float32
    bf16 = mybir.dt.bfloat16
    AF = mybir.ActivationFunctionType

    nc = bacc.Bacc("TRN2", target_bir_lowering=False)

    fp8 = mybir.dt.float8e4
    x_in = nc.dram_tensor("x_in", [L, D], bf16, kind="ExternalInput")
    wq_d = nc.dram_tensor("wq", [D, H * HD], fp8, kind="ExternalInput")
    wk_d = nc.dram_tensor("wk", [D, H * HD], fp8, kind="ExternalInput")
    wv_d = nc.dram_tensor("wv", [D, H * HD], fp8, kind="ExternalInput")
    wo_d = nc.dram_tensor("wo", [H * HD, D], fp8, kind="ExternalInput")
    w1_d = nc.dram_tensor("w1", [D, MLP], fp8, kind="ExternalInput")
    w2_d = nc.dram_tensor("w2", [MLP, D], fp8, kind="ExternalInput")
    b1_d = nc.dram_tensor("b1", [MLP], f32, kind="ExternalInput")
    b2_d = nc.dram_tensor("b2", [D], f32, kind="ExternalInput")
    if bias1:
        ln1b_d = nc.dram_tensor("ln1b", [D], f32, kind="ExternalInput")
    pad_d = nc.dram_tensor("padm", [P, M], f32, kind="ExternalInput")
    out_d = nc.dram_tensor("out", [L, D], bf16, kind="ExternalOutput")

    NT = L // P
    DC = D // P
    HP = H // 2
    MC = MLP // P
    EPS = 1e-6

    with tile.TileContext(nc) as tc, ExitStack() as ctx:
        const = ctx.enter_context(tc.tile_pool(name="const", bufs=1))
        big = ctx.enter_context(tc.tile_pool(name="big", bufs=1))
        resid = ctx.enter_context(tc.tile_pool(name="resid", bufs=2))
        tok = ctx.enter_context(tc.tile_pool(name="tok", bufs=2))
        small = ctx.enter_context(tc.tile_pool(name="small", bufs=4))
        rows = ctx.enter_context(tc.tile_pool(name="rows", bufs=1))
        attnp = ctx.enter_context(tc.tile_pool(name="attnp", bufs=10))
        vkmp = ctx.enter_context(tc.tile_pool(name="vkmp", bufs=2))
        bcast = ctx.enter_context(tc.tile_pool(name="bcast", bufs=1))
        normt = ctx.enter_context(tc.tile_pool(name="normt", bufs=2))
        psU = ctx.enter_context(tc.tile_pool(name="psU", bufs=4,
                                              space="PSUM"))
        psAV = ctx.enter_context(tc.tile_pool(name="psAV", bufs=2, space="PSUM"))

        xts = []
        for t in range(NT):
            xt = tok.tile([P, D], bf16, tag="xt", bufs=NT,
                          name=f"xt{t}")
            nc.sync.dma_start(xt[:], x_in[t * P:(t + 1) * P, :])
            xts.append(xt)

        id_bf = const.tile([P, P], bf16)
        make_identity(nc, id_bf[:])

        def load_w(dram, kdim, ndim):
            t = const.tile([P, kdim // P, ndim], fp8, tag=dram.name)
            nc.sync.dma_start(t[:], dram.rearrange("(c p) n -> p c n", p=P))
            return t

        wq = load_w(wq_d, D, H * HD)
        wk = load_w(wk_d, D, H * HD)
        wv = load_w(wv_d, D, H * HD)
        wo = load_w(wo_d, H * HD, D)
        w1 = load_w(w1_d, D, MLP)
        w2 = load_w(w2_d, MLP, D)

        b1c = const.tile([P, MC], f32)
        nc.sync.dma_start(b1c[:], b1_d.rearrange("(c p) -> p c", p=P))
        b2c = const.tile([P, DC], f32)
        nc.sync.dma_start(b2c[:], b2_d.rearrange("(c p) -> p c", p=P))
        padm = const.tile([P, M], f32)
        nc.sync.dma_start(padm[:], pad_d[:])
        if bias1:
            ln1bB = const.tile([P, D], f32)
            nc.sync.dma_start(ln1bB[:],
                              ln1b_d[None, :].to_broadcast((P, D)))
        eps_col = const.tile([P, 1], f32)
        nc.vector.memset(eps_col[:], EPS)
        # ones row at partition 64: K=1 stationary for broadcasting the
        # 1/denom rows (which live at partition 64 of each acc tile)
        selb = const.tile([65, P], bf16)
        nc.vector.memset(selb[64:65, :], 1.0)
        onesb = const.tile([1, P], bf16)
        nc.vector.memset(onesb[:], 1.0)
        ones_colb = const.tile([P, 1], bf16)
        nc.vector.memset(ones_colb[:], 1.0)

        xT = big.tile([P, DC, L], fp8, tag="xT")
        inT = resid.tile([P, DC, L], bf16, tag="res")
        qT = big.tile([P, HP, L], bf16, tag="qT")
        kT = big.tile([P, HP, L], bf16, tag="kT")
        vT = big.tile([P, HP, L], bf16, tag="vT")
        y1T = big.tile([P, MC, L], fp8, tag="y1T")
        # outT / ln2T are allocated later with tags "xT" / "qT", reusing the
        # slots of xT (dead after QKV) and qT (dead after attention).

        # ---- LN1 (token-major) + transposes ----
        # ln1_scale is folded into Wq/Wk/Wv host-side; when fold_b1 the bias
        # is b/s added pre-projection (exact), and when the bias is zero it
        # vanishes. Stats: sum(x) and sum(x^2) via Act accumulators (keeps
        # the DVE off the critical path); var = E[x^2] - mu^2.
        f32r = mybir.dt.float32r

        def ln1_tile(t):
            xt = xts[t]
            for c in range(DC):
                pf = psU.tile([P, P], bf16, tag="u", name=f"tpb{t}_{c}")
                nc.tensor.transpose(pf[:], xt[:, c * P:(c + 1) * P],
                                    id_bf[:])
                nc.vector.tensor_copy(inT[:, c, t * P:(t + 1) * P], pf[:])
            sumx = small.tile([P, 1], f32, tag="sumx")
            cpy = tok.tile([P, D], bf16, tag="cent")
            nc.vector.scalar_tensor_tensor(
                out=cpy[:], in0=xt[:], scalar=1.0, in1=xt[:],
                op0=mybir.AluOpType.mult, op1=mybir.AluOpType.max,
                accum_out=sumx[:])
            sq = tok.tile([P, D], bf16, tag="sq")
            sumsq = small.tile([P, 1], f32, tag="sumsq")
            nc.scalar.activation(sq[:], xt[:], AF.Square, accum_out=sumsq[:])
            # stats smalls kept on DVE with ONE Act hop (Sqrt) to avoid
            # per-tile cross-engine ping-pong on the in-order queues
            nmean = small.tile([P, 1], f32, tag="nmean")
            nc.vector.tensor_scalar(
                nmean[:], sumx[:], -1.0 / D, 0.0,
                op0=mybir.AluOpType.mult, op1=mybir.AluOpType.add)
            mu2 = small.tile([P, 1], f32, tag="mu2s")
            nc.vector.tensor_mul(mu2[:], nmean[:], nmean[:])
            varb = small.tile([P, 1], f32, tag="varb")
            nc.vector.scalar_tensor_tensor(
                out=varb[:], in0=sumsq[:], scalar=1.0 / D, in1=mu2[:],
                op0=mybir.AluOpType.mult, op1=mybir.AluOpType.subtract)
            rstd = small.tile([P, 1], f32, tag="rstd")
            nc.scalar.activation(rstd[:], varb[:], AF.Sqrt, bias=eps_col[:])
            nc.vector.reciprocal(rstd[:], rstd[:])
            nmr = small.tile([P, 1], f32, tag="nmr")
            nc.vector.tensor_mul(nmr[:], nmean[:], rstd[:])
            xnb = tok.tile([P, D], bf16, tag="xnb")
            with nc.allow_low_precision(reason="qkv in fp8: rel tol 2e-2"):
                if bias1:
                    xn = tok.tile([P, D], f32, tag="xn")
                    nc.vector.tensor_scalar(
                        xn[:], xt[:], rstd[:], nmr[:],
                        op0=mybir.AluOpType.mult, op1=mybir.AluOpType.add)
                    nc.vector.tensor_add(xnb[:], xn[:], ln1bB[:])
                else:
                    nc.vector.tensor_scalar(
                        xnb[:], xt[:], rstd[:], nmr[:],
                        op0=mybir.AluOpType.mult, op1=mybir.AluOpType.add)
            for c in range(DC):
                pt = psU.tile([P, P], bf16, tag="u", name=f"tpa{t}_{c}")
                nc.tensor.transpose(pt[:], xnb[:, c * P:(c + 1) * P],
                                    id_bf[:])
                with nc.allow_low_precision(reason="qkv in fp8"):
                    nc.scalar.copy(xT[:, c, t * P:(t + 1) * P], pt[:])

        # ---- QKV (V first so the vkm prebuild below overlaps K/Q work) ----
        # fp8 DoubleRow: K=512 contracts as 2 groups of (128 partitions x 2).
        # Emission is split by token half and interleaved with the LN1 tiles
        # so the nh=0 projections enter the PE queue as soon as tiles 0-3 are
        # transposed, instead of sitting behind all 64 LN1 transposes.
        DR = mybir.MatmulPerfMode.DoubleRow

        for t in range(NT):
            ln1_tile(t)
        for w_sb, dstT in ((wv, vT), (wk, kT), (wq, qT)):
            for hp in range(HP):
                for nh in range(2):
                    ps = psU.tile([P, 512], f32, tag="u")
                    for c2 in range(DC // 2):
                        nc.tensor.matmul(
                            ps[:],
                            w_sb[:, 2 * c2:2 * c2 + 2, hp * P:(hp + 1) * P],
                            xT[:, 2 * c2:2 * c2 + 2,
                               nh * 512:(nh + 1) * 512],
                            start=(c2 == 0), stop=(c2 == DC // 2 - 1),
                            perf_mode=DR)
                    if nh == 0:
                        nc.vector.tensor_copy(
                            dstT[:, hp, nh * 512:(nh + 1) * 512], ps[:])
                    else:
                        nc.scalar.copy(
                            dstT[:, hp, nh * 512:(nh + 1) * 512], ps[:])

        # ---- attention ----
        # psum row halves are indexed by he (head within pair); every matmul
        # chain into an acc tile keeps ONE base partition (HW requirement:
        # accumulating from different tile_positions crashes the device).
        # vkm prebuild for ALL head pairs (overlaps the K/Q projections):
        # vkm[he*64+p, j, 0:64] = v[key j*64+p, hd] of head 2hp+he;
        # col 64 = ones (softmax denominator; zeroed for padded keys).
        # Each 64x64 transpose targets its own partition half directly, so a
        # single batched copy moves all M blocks for both halves.
        vkms = []
        for hp in range(HP):
            vkm = vkmp.tile([P, M, 65], bf16, tag="vkm", bufs=HP,
                            name=f"vkm{hp}")
            pt = psU.tile([P, M, 64], bf16, tag="u", name=f"tpv{hp}")
            for j in range(M):
                nc.tensor.transpose(pt[0:64, j, :],
                                    vT[0:64, hp, j * 64:(j + 1) * 64],
                                    id_bf[0:64, 0:64])
                nc.tensor.transpose(pt[64:128, j, :],
                                    vT[64:128, hp, j * 64:(j + 1) * 64],
                                    id_bf[64:128, 64:128])
            # ones column is 32.0: v is x32 (fp8 weight scaling), so denom
            # and av keep the same scale and the ratio is exact
            nc.vector.tensor_copy(vkm[:, :, 0:64], pt[:, :, :])
            nc.vector.memset(vkm[:, :, 64:65], 32.0)
            for j in range(M):
                nc.vector.tensor_scalar_mul(vkm[:, j, :], vkm[:, j, :],
                                            padm[:, j:j + 1])
            vkms.append(vkm)

        outT = big.tile([P, HP, L], fp8, tag="xT")

        def make_norm(hp, acc):
            # normalize: out = acc[0:64] / acc[64]. bf16 reciprocals, then a
            # K=1 bf16 ones-matmul broadcasts each 1/d row across 64
            # partitions. he1's rows move to outT's upper half by gpsimd DMA.
            # Emission is deferred into the next hp's first chunk so the PE
            # queue has independent score work while the reciprocals land.
            def norm():
                recm = [rows.tile([65, L], bf16, tag=f"rec{he}",
                                  name=f"rec{hp}_{he}") for he in range(2)]
                with nc.allow_low_precision(reason="1/denom bf16"):
                    nc.vector.reciprocal(recm[0][64:65, :], acc[0][64:65, :])
                    nc.vector.reciprocal(recm[1][64:65, :], acc[1][64:65, :])
                nt = normt.tile([64, L], fp8, tag="nt")
                for nh in range(2):
                    sl = slice(nh * 512, (nh + 1) * 512)
                    for he in range(2):
                        recB = psU.tile([64, 512], f32, tag="u",
                                        name=f"recB{hp}_{nh}_{he}")
                        nc.tensor.matmul(recB[:], selb[64:65, 0:64],
                                         recm[he][64:65, sl],
                                         start=True, stop=True)
                        recS = bcast.tile([64, 512], f32, tag="recB",
                                          bufs=2)
                        nc.scalar.copy(recS[:], recB[:])
                        with nc.allow_low_precision(reason="attn out fp8"):
                            if he == 0:
                                nc.vector.tensor_mul(outT[0:64, hp, sl],
                                                     acc[0][0:64, sl],
                                                     recS[:])
                            else:
                                nc.vector.tensor_mul(nt[:, sl],
                                                     acc[1][0:64, sl],
                                                     recS[:])
                    nc.gpsimd.dma_start(outT[64:128, hp, sl], nt[:, sl])
            return norm

        def make_av(acc, vkm, att, chunk, j):
            # AV emission is software-pipelined one chunk behind the scores
            # so the in-order PE queue never head-of-line blocks on an exp
            def av():
                for he in range(2):
                    po = he * 64
                    col = 0
                    for (i0, n) in _runs_b(chunk):
                        nc.tensor.matmul(
                            acc[he][0:65, i0 * 64:(i0 + n) * 64],
                            vkm[po:po + 64, j, :],
                            att[po:po + 64, col * 64:(col + n) * 64],
                            start=(j == 0), stop=(j == M - 1))
                        col += n
            return av

        pending_norm = None
        pending_av = None
        for hp in range(HP):
            vkm = vkms[hp]
            acc = [psAV.tile([65, L], f32, tag="avacc", name=f"acc{hp}_{he}")
                   for he in range(2)]
            nchunk = 0
            for j in range(M):
                csizes = _chunk_sizes(len(Q[j]))
                for ci, csz in enumerate(csizes):
                    s0 = 8 * ci
                    chunk = Q[j][s0:s0 + csz]
                    pssc = psU.tile([P, 512], f32, tag="u",
                                     name=f"sc{hp}_{j}_{ci}")
                    for he in range(2):
                        po = he * 64
                        col = 0
                        for (i0, n) in _runs(chunk):
                            nc.tensor.matmul(
                                pssc[po:po + 64, col * 64:(col + n) * 64],
                                kT[po:po + 64, hp, j * 64:(j + 1) * 64],
                                qT[po:po + 64, hp, i0 * 64:(i0 + n) * 64],
                                start=True, stop=True)
                            col += n
                    att = attnp.tile([P, 512], bf16, tag="att",
                                     name=f"att{hp}_{j}_{ci}")
                    # scores carry the x32 q and x32 k scalings: exp
                    # rescales by 2^-10 exactly
                    nc.scalar.activation(att[:, 0:csz * 64],
                                         pssc[:, 0:csz * 64], AF.Exp,
                                         scale=2.0 ** -10)
    # flush order matters at hp boundaries: the previous
                    # hp's last AV must hit the PE queue before its norm
                    # broadcasts (which wait on the DVE reciprocal of acc);
                    # the norm itself waits one more chunk so two chunks of
                    # scores cover the 2x 1.2us serialized DVE reciprocals
                    if pending_av is not None:
                        pending_av()
                        pending_av = None
                    if pending_norm is not None and nchunk >= 2:
                        pending_norm()
                        pending_norm = None
                    pending_av = make_av(acc, vkm, att, chunk, j)
                    nchunk += 1
            pending_norm = make_norm(hp, acc)
        pending_av()
        pending_norm()

        # ---- Wo + residual ----
        xrT = resid.tile([P, DC, L], bf16, tag="res")
        for nh in range(2):
            for dc in range(DC):
                sl = slice(nh * 512, (nh + 1) * 512)
                ps = psU.tile([P, 512], f32, tag="u")
                for c2 in range(HP // 2):
                    nc.tensor.matmul(
                        ps[:],
                        wo[:, 2 * c2:2 * c2 + 2, dc * P:(dc + 1) * P],
                        outT[:, 2 * c2:2 * c2 + 2, sl],
                        start=(c2 == 0), stop=(c2 == HP // 2 - 1),
                        perf_mode=DR)
                with nc.allow_low_precision(reason="xr bf16: tol 2e-2"):
                    nc.vector.scalar_tensor_tensor(
                        out=xrT[:, dc, sl], in0=ps[:], scalar=2.0 ** -5,
                        in1=inT[:, dc, sl],
                        op0=mybir.AluOpType.mult, op1=mybir.AluOpType.add)

        # ---- LN2 (transposed space; stats via ones-matmul) ----
        mu = rows.tile([1, L], f32, tag="mu")
        msq = rows.tile([1, L], f32, tag="msq")
        mub = rows.tile([1, L], bf16, tag="mub")
        msqb = rows.tile([1, L], bf16, tag="msqb")
        for nh in range(2):
            ps_s = psU.tile([1, 512], f32, tag="u", name=f"st_s{nh}")
            ps_q = psU.tile([1, 512], f32, tag="u", name=f"st_q{nh}")
            for dc in range(DC):
                sl = slice(nh * 512, (nh + 1) * 512)
                sqc = tok.tile([P, 512], bf16, tag="sqc")
                with nc.allow_low_precision(reason="ln2 stats in bf16"):
                    nc.scalar.activation(sqc[:], xrT[:, dc, sl], AF.Square)
                nc.tensor.matmul(ps_s[:], ones_colb[:], xrT[:, dc, sl],
                                 start=(dc == 0), stop=(dc == DC - 1))
                nc.tensor.matmul(ps_q[:], ones_colb[:], sqc[:],
                                 start=(dc == 0), stop=(dc == DC - 1))
            sl = slice(nh * 512, (nh + 1) * 512)
            nc.scalar.mul(mu[0:1, sl], ps_s[:], 1.0 / D)
            nc.scalar.mul(msq[0:1, sl], ps_q[:], 1.0 / D)
            # msq <- rstd = 1/sqrt(msq - mu^2 + eps) per half, in place
            mu2 = rows.tile([1, L], f32, tag="mu2")
            nc.scalar.activation(mu2[0:1, sl], mu[0:1, sl], AF.Square)
            nc.vector.tensor_sub(msq[0:1, sl], msq[0:1, sl], mu2[0:1, sl])
            nc.scalar.activation(msq[0:1, sl], msq[0:1, sl], AF.Sqrt,
                                 bias=eps_col[0:1, :])
            with nc.allow_low_precision(reason="ln2 stat rows bf16"):
                nc.vector.reciprocal(msqb[0:1, sl], msq[0:1, sl])
                nc.vector.tensor_copy(mub[0:1, sl], mu[0:1, sl])
        ln2T = big.tile([P, DC, L], fp8, tag="qT")
        for nh in range(2):
            sl = slice(nh * 512, (nh + 1) * 512)
            muB = psU.tile([P, 512], f32, tag="u", name=f"muB{nh}")
            rstdB = psU.tile([P, 512], f32, tag="u", name=f"rstdB{nh}")
            nc.tensor.matmul(muB[:], onesb[0:1, :], mub[0:1, sl],
                             start=True, stop=True)
            nc.tensor.matmul(rstdB[:], onesb[0:1, :], msqb[0:1, sl],
                             start=True, stop=True)
            muS = bcast.tile([P, 512], bf16, tag="muS", bufs=2)
            rstdS = bcast.tile([P, 512], bf16, tag="rstdS", bufs=2)
            with nc.allow_low_precision(reason="ln2 bcast bf16"):
                nc.scalar.copy(muS[:], muB[:])
                nc.scalar.copy(rstdS[:], rstdB[:])
            for dc in range(DC):
                t1 = tok.tile([P, 512], bf16, tag="sqc")
                with nc.allow_low_precision(reason="mlp in fp8"):
                    nc.vector.tensor_sub(t1[:], xrT[:, dc, sl], muS[:])
                    nc.vector.tensor_mul(ln2T[:, dc, sl], t1[:], rstdS[:])

        # ---- MLP (nh-outer so each half reaches the store pipeline early) ----
        # foutT/ost/out are bf16: the final sum is dominated by the f32
        # residual path, so the bf16 rounding adds ~0.1% RMS (tol is 2e-2)
        foutT = resid.tile([P, DC, L], bf16, tag="res")
        dmaq = [nc.sync, nc.scalar, nc.gpsimd]
        for nh in range(2):
            for mc in range(MC):
                ps = psU.tile([P, 512], f32, tag="u")
                for c2 in range(DC // 2):
                    nc.tensor.matmul(
                        ps[:], w1[:, 2 * c2:2 * c2 + 2, mc * P:(mc + 1) * P],
                        ln2T[:, 2 * c2:2 * c2 + 2,
                             nh * 512:(nh + 1) * 512],
                        start=(c2 == 0), stop=(c2 == DC // 2 - 1),
                        perf_mode=DR)
                with nc.allow_low_precision(reason="mlp hidden fp8"):
                    nc.scalar.activation(y1T[:, mc, nh * 512:(nh + 1) * 512],
                                         ps[:], AF.Gelu_apprx_tanh,
                                         bias=b1c[:, mc:mc + 1],
                                         scale=2.0 ** -5)
            for dc in range(DC):
                ps = psU.tile([P, 512], f32, tag="u")
                for c2 in range(MC // 2):
                    nc.tensor.matmul(
                        ps[:], w2[:, 2 * c2:2 * c2 + 2, dc * P:(dc + 1) * P],
                        y1T[:, 2 * c2:2 * c2 + 2,
                            nh * 512:(nh + 1) * 512],
                        start=(c2 == 0), stop=(c2 == MC // 2 - 1),
                        perf_mode=DR)
                sl = slice(nh * 512, (nh + 1) * 512)
                with nc.allow_low_precision(reason="final out bf16"):
                    if bias2:
                        t2 = tok.tile([P, 512], f32, tag="sqc",
                                      name=f"t2_{dc}_{nh}")
                        nc.vector.tensor_scalar(
                            t2[:], ps[:], 2.0 ** -5, b2c[:, dc:dc + 1],
                            op0=mybir.AluOpType.mult,
                            op1=mybir.AluOpType.add)
                        nc.vector.tensor_add(foutT[:, dc, sl], t2[:],
                                             xrT[:, dc, sl])
                    else:
                        nc.vector.scalar_tensor_tensor(
                            out=foutT[:, dc, sl], in0=ps[:],
                            scalar=2.0 ** -5, in1=xrT[:, dc, sl],
                            op0=mybir.AluOpType.mult,
                            op1=mybir.AluOpType.add)

            # transpose this half back to token-major, store full-width
            # [128, D] slabs (one 1KB/partition DMA per tile, queues cycled)
            for t in range(nh * NT // 2, (nh + 1) * NT // 2):
                ost = normt.tile([P, D], bf16, tag="ost")
                for dc in range(DC):
                    pf = psU.tile([P, P], bf16, tag="u",
                                   name=f"tpo{dc}_{t}")
                    nc.tensor.transpose(pf[:],
                                        foutT[:, dc, t * P:(t + 1) * P],
                                        id_bf[:])
                    if dc % 2 == 0:
                        nc.vector.tensor_copy(ost[:, dc * P:(dc + 1) * P],
                                              pf[:])
                    else:
                        nc.scalar.copy(ost[:, dc * P:(dc + 1) * P], pf[:])
                dmaq[t % 3].dma_start(out_d[t * P:(t + 1) * P, :], ost[:])

    nc.compile()
    return nc


def kernel(**inputs):
    inputs = {k: np.asarray(v) for k, v in inputs.items()}
    rand_attn = inputs["rand_attn"].astype(np.int32)
    ln1s = inputs["ln1_scale"].astype(np.float32)
    ln1b = inputs["ln1_bias"].astype(np.float32)
    ln2s = inputs["ln2_scale"].astype(np.float32)
    ln2b = inputs["ln2_bias"].astype(np.float32)
    bias1 = bool(np.any(ln1b != 0.0))
    bias2 = bool(np.any(np.asarray(inputs["b2"]) != 0.0))
    key = (rand_attn.tobytes(), bias1, bias2)
    if key not in _kernel_cache:
        S = _attended_sets(rand_attn)
        Q = [[i for i in range(M) if j in S[i]] for j in range(M)]
        _kernel_cache[key] = _build_bass(S, Q, bias1, bias2)
    nc = _kernel_cache[key]

    x = inputs["inputs"].astype(np.float32)
    pm = np.asarray(inputs["padding_mask"]).astype(np.float32)
    # ln1_scale folds into the qkv projections (rows scaled); a nonzero
    # ln1_bias becomes a pre-projection add of b/s (exact). ln2 scale AND
    # bias fold into W1/b1 exactly: gelu(ln*s2+b2c)@W1+b1 =
    # gelu(ln@(diag(s2)W1) + (b2c@W1+b1)).
    # weights are scaled x32 before fp8 quantization (their ~0.02 magnitudes
    # sit in e4m3's denormal range otherwise); every x32 is compensated
    # downstream by an exact power-of-2 rescale inside the kernel.
    s1 = np.where(ln1s == 0.0, np.float32(1e-30), ln1s)[:, None]
    wq = _np_fp8(32.0 * s1 * inputs["Wq"].reshape(D, H * HD) / np.sqrt(HD))
    wk = _np_fp8(32.0 * s1 * inputs["Wk"].reshape(D, H * HD))
    wv = _np_fp8(32.0 * s1 * inputs["Wv"].reshape(D, H * HD))
    wo = _np_fp8(32.0 * inputs["Wo"].reshape(H * HD, D))
    w1f = inputs["W1"].astype(np.float32)
    w1 = _np_fp8(32.0 * ln2s[:, None] * w1f)
    w2 = _np_fp8(32.0 * inputs["W2"])
    b1f = inputs["b1"].astype(np.float32) + ln2b @ w1f
    common = dict(
        wq=wq, wk=wk, wv=wv, wo=wo, w1=w1, w2=w2,
        b1=b1f.astype(np.float32),
        b2=inputs["b2"].astype(np.float32),
    )
    if bias1:
        common["ln1b"] = (ln1b / s1[:, 0]).astype(np.float32)
    in_maps = []
    for c in range(NCORES):
        pj = pm[c, :, 0].reshape(M, BLK).T          # [64, M]
        padm = np.concatenate([pj, pj], axis=0)      # both he row halves
        in_maps.append(dict(common, x_in=_np_bf16(x[c]),
                            padm=padm.astype(np.float32)))

    from concourse.bass_utils import run_bass_kernel_spmd
    res = run_bass_kernel_spmd(nc, in_maps, core_ids=list(range(NCORES)))
    return np.stack([np.asarray(res.results[c]["out"], np.float32)
                     for c in range(NCORES)], axis=0)

